# revision 1
# baseline (speedup 1.0000x reference)
"""MoE layer (8 experts, top-2, capacity 2560) on 8 Trainium2 NeuronCores.

Expert-parallel: one expert per core. Host does gating/routing (the
data-dependent "sharding"/dispatch step) and the final weighted combine;
each core runs the dense expert FFN  relu(buf @ w1 + b1) @ w2 + b2  for
its expert over the filled part of the capacity-padded dispatch buffer.

Device kernel (per core): the expert's filled rows are processed in
NCHUNK chunks of CHUNK tokens (chosen at runtime from the max expert
load, so padded all-zero capacity rows are not computed). Per chunk:
  layer 1: for each of 64 hidden tiles (128 rows of H), accumulate 16
    matmuls (contraction D=2048 in 128-tiles) into one PSUM bank, then
    ACT relu+bias into an SBUF-resident hidden tile [128, CHUNK] (bf16).
  layer 2: 4 sweeps of 4 output d-tiles; each sweep accumulates 64
    matmuls per d-tile (contraction H=8192) across 4 PSUM banks, then
    ACT copy+bias to SBUF and DMA out.
Weights stream from HBM (re-read once per chunk); activations stay in
SBUF. bf16 multiplies, fp32 PSUM accumulation.
"""

import math

import numpy as np
import ml_dtypes

import concourse.bacc as bacc
import concourse.mybir as mybir
import concourse.tile as tile
from concourse import bass_utils

F32 = mybir.dt.float32
BF16 = mybir.dt.bfloat16
AF = mybir.ActivationFunctionType

# Problem constants (from the reference module).
NUM_EXPERTS = 8
TOP_K = 2
D = 2048          # d_model
H = 8192          # d_hidden
B, S = 4, 2048
T = B * S         # 8192 tokens
CAP = 2560        # ceil(T*K/E * 1.25)

DT = 16           # d tiles (DT*128 == D)
HT = 64           # h tiles (HT*128 == H)
DQ = 4            # layer-2 sweeps (DQ * 4 d-tiles == DT)

_CACHE = {}


def _build_nc(nchunk, chunk):
    """nchunk must be even: chunks are processed in pairs so each
    streamed weight tile serves two chunks (halves weight traffic)."""
    assert nchunk % 2 == 0
    nc = bacc.Bacc("TRN2", target_bir_lowering=False, debug=False)
    bufx = nc.dram_tensor("bufx", [nchunk, 128, DT, chunk], BF16, kind="ExternalInput")
    w1x = nc.dram_tensor("w1x", [HT, 128, DT, 128], BF16, kind="ExternalInput")
    w2x = nc.dram_tensor("w2x", [8, HT // 2, 128, 2, 2, 128], BF16, kind="ExternalInput")
    b1x = nc.dram_tensor("b1x", [128, HT], F32, kind="ExternalInput")
    b2x = nc.dram_tensor("b2x", [128, DT], F32, kind="ExternalInput")
    outx = nc.dram_tensor("outx", [nchunk, DT, 128, chunk], F32, kind="ExternalOutput")

    with tile.TileContext(nc) as tc:
        with (
            tc.tile_pool(name="consts", bufs=1) as consts,
            tc.tile_pool(name="bufp", bufs=4) as bufp,
            tc.tile_pool(name="w1p", bufs=3) as w1p,
            tc.tile_pool(name="w2p", bufs=8) as w2p,
            tc.tile_pool(name="hp", bufs=2) as hp,
            tc.tile_pool(name="outp", bufs=4) as outp,
            tc.tile_pool(name="ps1", bufs=4, space="PSUM") as ps1,
            tc.tile_pool(name="ps2", bufs=4, space="PSUM") as ps2,
        ):
            b1_sb = consts.tile([128, HT], F32)
            b2_sb = consts.tile([128, DT], F32)
            nc.sync.dma_start(b1_sb[:], b1x[:])
            nc.sync.dma_start(b2_sb[:], b2x[:])

            for cp in range(nchunk // 2):
                ca, cb = 2 * cp, 2 * cp + 1
                buf_a = bufp.tile([128, DT, chunk], BF16, name=f"buf_a{cp}", tag="buf")
                buf_b = bufp.tile([128, DT, chunk], BF16, name=f"buf_b{cp}", tag="buf")
                nc.sync.dma_start(buf_a[:], bufx[ca])
                nc.sync.dma_start(buf_b[:], bufx[cb])
                hT_a = hp.tile([128, HT, chunk], BF16, name=f"hT_a{cp}", tag="hT")
                hT_b = hp.tile([128, HT, chunk], BF16, name=f"hT_b{cp}", tag="hT")

                # ---- layer 1: hT[ht] = relu(w1[:,ht]^T @ bufT + b1[ht]) ----
                for ht in range(HT):
                    w1_sb = w1p.tile([128, DT, 128], BF16)
                    nc.sync.dma_start(w1_sb[:], w1x[ht])
                    ps_a = ps1.tile([128, chunk], F32, name=f"ps_a{ht}", tag="ps1")
                    ps_b = ps1.tile([128, chunk], F32, name=f"ps_b{ht}", tag="ps1")
                    for dt in range(DT):
                        nc.tensor.matmul(
                            ps_a[:], w1_sb[:, dt, :], buf_a[:, dt, :],
                            start=(dt == 0), stop=(dt == DT - 1),
                        )
                        nc.tensor.matmul(
                            ps_b[:], w1_sb[:, dt, :], buf_b[:, dt, :],
                            start=(dt == 0), stop=(dt == DT - 1),
                        )
                    nc.scalar.activation(
                        hT_a[:, ht, :], ps_a[:], AF.Relu, bias=b1_sb[:, ht:ht + 1])
                    nc.scalar.activation(
                        hT_b[:, ht, :], ps_b[:], AF.Relu, bias=b1_sb[:, ht:ht + 1])

                # ---- layer 2: out[dt] = sum_ht w2[ht,dt]^T @ hT[ht] + b2 ----
                # 8 half-sweeps of 2 d-tiles x 2 chunks (4 PSUM banks each)
                for dh in range(8):
                    pss = [
                        ps2.tile([128, chunk], F32, name=f"pso_{dh}_{i}", tag="pso")
                        for i in range(4)
                    ]
                    for hpi in range(HT // 2):
                        w2_sb = w2p.tile([128, 2, 2, 128], BF16)
                        nc.sync.dma_start(w2_sb[:], w2x[dh, hpi])
                        for t in range(2):
                            ht = 2 * hpi + t
                            for i in range(2):
                                nc.tensor.matmul(
                                    pss[i][:], w2_sb[:, t, i, :], hT_a[:, ht, :],
                                    start=(ht == 0), stop=(ht == HT - 1),
                                )
                                nc.tensor.matmul(
                                    pss[2 + i][:], w2_sb[:, t, i, :], hT_b[:, ht, :],
                                    start=(ht == 0), stop=(ht == HT - 1),
                                )
                    for i in range(2):
                        dt = dh * 2 + i
                        for j, cc in ((0, ca), (2, cb)):
                            o_sb = outp.tile([128, chunk], F32)
                            nc.scalar.activation(
                                o_sb[:], pss[i + j][:], AF.Identity,
                                bias=b2_sb[:, dt:dt + 1])
                            nc.sync.dma_start(outx[cc, dt], o_sb[:])
    nc.compile()
    return nc


def _get_nc(nchunk, chunk):
    key = (nchunk, chunk)
    if key not in _CACHE:
        _CACHE[key] = _build_nc(nchunk, chunk)
    return _CACHE[key]


def _pick_shape(max_rows):
    """Pick (nchunk, chunk) with nchunk*chunk >= max_rows, chunk a
    multiple of 32 and <= 512, minimizing computed rows (tie: fewer
    chunks -> less weight streaming)."""
    best = None
    for nchunk in range(4, 21, 2):      # even: chunks processed in pairs
        chunk = int(math.ceil(max_rows / nchunk / 32)) * 32
        if chunk > 512 or chunk < 128:
            continue
        rows = nchunk * chunk
        cost = (rows, nchunk)
        if best is None or cost < best[0]:
            best = (cost, nchunk, chunk)
    if best is None:          # tiny loads: floor at 4 chunks of 128
        return 4, 128
    return best[1], best[2]


def _route(x_flat, gating_w):
    """Gating softmax + top-k, replicating the reference's jax ops (same
    backend) so routing decisions match bitwise. Falls back to float64
    numpy if jax is unavailable."""
    try:
        import jax
        import jax.numpy as jnp

        gates = jax.nn.softmax(jnp.asarray(x_flat) @ jnp.asarray(gating_w), axis=-1)
        topk_w, topk_idx = jax.lax.top_k(gates, TOP_K)
        norm_w = topk_w / (jnp.sum(topk_w, axis=-1, keepdims=True) + 1e-8)
        return (np.asarray(topk_idx, dtype=np.int64),
                np.asarray(norm_w, dtype=np.float32))
    except Exception:
        logits = x_flat.astype(np.float64) @ gating_w.astype(np.float64)
        m = logits.max(axis=-1, keepdims=True)
        e = np.exp(logits - m)
        gates = (e / e.sum(axis=-1, keepdims=True)).astype(np.float32)
        # top-k with ties broken toward lower index, descending order
        order = np.argsort(-gates, axis=-1, kind="stable")
        topk_idx = order[:, :TOP_K]
        topk_w = np.take_along_axis(gates, topk_idx, axis=-1)
        norm_w = topk_w / (topk_w.sum(axis=-1, keepdims=True) + 1e-8)
        return topk_idx.astype(np.int64), norm_w.astype(np.float32)


def kernel(x, gating_w, w1, b1, w2, b2, **run_kwargs):
    x = np.ascontiguousarray(np.asarray(x, dtype=np.float32))
    gating_w = np.asarray(gating_w, dtype=np.float32)
    w1 = np.asarray(w1, dtype=np.float32)
    b1 = np.asarray(b1, dtype=np.float32)
    w2 = np.asarray(w2, dtype=np.float32)
    b2 = np.asarray(b2, dtype=np.float32)

    x_flat = x.reshape(T, D)

    # ---- routing (host) ----
    topk_idx, norm_w = _route(x_flat, gating_w)
    flat_e = topk_idx.reshape(-1)                       # [T*K]
    flat_t = np.repeat(np.arange(T, dtype=np.int64), TOP_K)
    flat_w = norm_w.reshape(-1)

    onehot = (flat_e[:, None] == np.arange(NUM_EXPERTS)[None, :]).astype(np.int32)
    pos_all = np.cumsum(onehot, axis=0) - 1
    position = pos_all[np.arange(T * TOP_K), flat_e]
    valid = position < CAP

    # Only the filled rows of each expert's capacity buffer need compute.
    counts = np.bincount(flat_e[valid], minlength=NUM_EXPERTS)
    max_rows = int(min(max(int(counts.max()), 128), CAP))
    nchunk, chunk = _pick_shape(max_rows)
    nrows = nchunk * chunk                              # >= max filled row

    # ---- dispatch (host side of the "all-to-all") ----
    buf = np.zeros((NUM_EXPERTS, nrows, D), dtype=np.float32)
    buf[flat_e[valid], position[valid]] = x_flat[flat_t[valid]]

    # ---- per-core input packing ----
    in_maps = []
    for e in range(NUM_EXPERTS):
        bufx = (buf[e].reshape(nchunk, chunk, DT, 128).transpose(0, 3, 2, 1)
                .astype(ml_dtypes.bfloat16))
        w1x = (w1[e].reshape(DT, 128, HT, 128).transpose(2, 1, 0, 3)
               .astype(ml_dtypes.bfloat16))
        w2x = (w2[e].reshape(HT // 2, 2, 128, 8, 2, 128)
               .transpose(3, 0, 2, 1, 4, 5)
               .astype(ml_dtypes.bfloat16))
        b1x = np.ascontiguousarray(b1[e].reshape(HT, 128).T)
        b2x = np.ascontiguousarray(b2[e].reshape(DT, 128).T)
        in_maps.append({
            "bufx": np.ascontiguousarray(bufx),
            "w1x": np.ascontiguousarray(w1x),
            "w2x": np.ascontiguousarray(w2x),
            "b1x": b1x, "b2x": b2x,
        })

    # ---- run expert FFNs on the 8 cores ----
    nc = _get_nc(nchunk, chunk)
    res = bass_utils.run_bass_kernel_spmd(
        nc, in_maps, core_ids=list(range(NUM_EXPERTS)), **run_kwargs)
    if run_kwargs.get("trace"):
        _CACHE["last_results"] = res

    out_all = np.empty((NUM_EXPERTS, nrows, D), dtype=np.float32)
    for e in range(NUM_EXPERTS):
        out_all[e] = (res.results[e]["outx"].transpose(0, 3, 1, 2)
                      .reshape(nrows, D))

    # ---- combine (host side of the "all-to-all" + weighted scatter-add) ----
    pos_g = np.minimum(position, nrows - 1)             # clamped rows get weight 0
    gathered = out_all[flat_e, pos_g]                   # [T*K, D]
    w_eff = np.where(valid, flat_w, 0.0).astype(np.float32)
    out_flat = (gathered * w_eff[:, None]).reshape(T, TOP_K, D).sum(axis=1)
    return out_flat.reshape(B, S, D).astype(np.float32)

